# revision 8
# baseline (speedup 1.0000x reference)
"""Trainium2 Bass kernel for nn_DifferentiableFeatureExtractor.

Strategy (8 NeuronCores, shard T=1048576 along time):
  - per-core extended domain EXT = S + 6144 halo = 137216 = 128 partitions x 1072
  - each partition holds a contiguous 1072-bar chunk plus a 256-bar AP halo
    (tile [128, 1328]); host supplies a 256-bar lead-in so partition 0's halo
    is real data (clamp-padded at the global left edge like the reference)
  - 20 truncated-EMA convs as fp32 recurrences: y = a*(s - c^K s[t-K]) with
    s from tensor_tensor_scan (2-pass blocked scan, partition carry chained
    via shifted end-columns; c^CH second-order term kept when significant)
  - sliding max/min via log-doubling with shifted APs (halo-local)
  - rolling std via anchored windowed sums (doubling)
  - BARSLAST/MA_DYNAMIC as segmented scans (reset at cross events) with
    affine partition-carry chains; exact whenever the previous cross lies
    within the 6144-bar halo (diag output flags violations -> host fallback)
All heavy compute runs on device; host only shards, gathers, patches the
17 reference partial-window std bars, and handles the (never-observed)
diag fallback.
"""
import math

import numpy as np

import concourse.bacc as bacc
import concourse.mybir as mybir
from concourse import tile as ctile
from concourse.bass_utils import run_bass_kernel_spmd

F32 = mybir.dt.float32
Alu = mybir.AluOpType
Act = mybir.ActivationFunctionType

T = 1048576
NCORES = 8
S = T // NCORES            # 131072
P = 128
CH = 1072                  # chunk cols per partition
HP = 256                   # per-partition halo cols
W = HP + CH                # 1328
EXT = P * CH               # 137216
HALO = EXT - S             # 6144
DLEN = HP + EXT            # 137472
C0 = HP                    # chunk start col
NROWS = 30

# static truncation lengths (depend only on the reference's constant ALPHAS)
KS = [72, 72, 72, 286, 286, 286, 559, 89, 54, 47, 40, 30, 130, 30,
      30, 30, 30, 37, 37, 37]


class KB:
    """kernel builder with a simple big-tile freelist"""

    def __init__(self, alphas, anchor):
        self.alphas = [float(a) for a in alphas]
        self.anchor = float(anchor)
        nc = bacc.Bacc(None, target_bir_lowering=False)
        self.nc = nc
        self.DC = nc.dram_tensor("DC", [DLEN], F32, kind="ExternalInput")
        self.DH = nc.dram_tensor("DH", [DLEN], F32, kind="ExternalInput")
        self.DL = nc.dram_tensor("DL", [DLEN], F32, kind="ExternalInput")
        self.OUT = nc.dram_tensor("OUT", [NROWS * S], F32, kind="ExternalOutput")
        self.DIAG = nc.dram_tensor("DIAG", [2], F32, kind="ExternalOutput")
        self.free_big = []
        self.n_big = 0
        self.free_small = []
        self.n_small = 0
        self.free_row = []
        self.n_row = 0

    # ---- tile management ----
    def big(self):
        if self.free_big:
            return self.free_big.pop()
        t = self.pool.tile([P, W], F32, tag=f"big{self.n_big}")
        self.n_big += 1
        return t

    def rel(self, *ts):
        for t in ts:
            self.free_big.append(t)

    def small(self):
        if self.free_small:
            return self.free_small.pop()
        t = self.spool.tile([P, 1], F32, tag=f"small{self.n_small}")
        self.n_small += 1
        return t

    def rels(self, *ts):
        for t in ts:
            self.free_small.append(t)

    def row(self):
        if self.free_row:
            return self.free_row.pop()
        t = self.spool.tile([1, P], F32, tag=f"row{self.n_row}")
        self.n_row += 1
        return t

    def relr(self, *ts):
        for t in ts:
            self.free_row.append(t)

    # ---- IO ----
    def load_series(self, dram):
        nc = self.nc
        t = self.big()
        nc.sync.dma_start(
            out=t[:, C0:W],
            in_=dram[HP:DLEN].rearrange("(p w) -> p w", p=P, w=CH),
        )
        nc.sync.dma_start(
            out=t[:, 0:HP],
            in_=dram[0:EXT].rearrange("(p w) -> p w", p=P, w=CH)[:, 0:HP],
        )
        return t

    def store_row(self, r, t):
        nc = self.nc
        nc.sync.dma_start(
            out=self.OUT[r * S : r * S + 288].rearrange("(p w) -> p w", p=1, w=288),
            in_=t[5:6, 1040:W],
        )
        nc.sync.dma_start(
            out=self.OUT[r * S + 288 : (r + 1) * S].rearrange(
                "(p w) -> p w", p=122, w=CH
            ),
            in_=t[6:P, C0:W],
        )

    # ---- building blocks ----
    def ema(self, xt, i, scale=1.0):
        """truncated EMA of xt (valid on chunk cols) -> new tile valid [248, W).
        Output scaled by `scale`."""
        nc = self.nc
        a = self.alphas[i]
        K = KS[i]
        c = 1.0 - a
        cF = float(c) ** CH
        cK = float(c) ** K

        cb = self.small()
        nc.vector.memset(cb[:, :], c)
        s = self.big()
        # scan1 (gpsimd): local scan, init 0
        nc.vector.tensor_tensor_scan(
            out=s[:, C0:W], data0=cb[:, 0:1].broadcast_to([P, CH]),
            data1=xt[:, C0:W], initial=0.0, op0=Alu.mult, op1=Alu.add,
        )
        # partition carry = E[p-1] (+ cF*E[p-2] when significant)
        ecol = self.small()
        nc.vector.memset(ecol[:, :], 0.0)
        nc.sync.dma_start(out=ecol[1:P, 0:1], in_=s[0 : P - 1, W - 1 : W])
        if cF > 1e-10:
            e2 = self.small()
            nc.vector.memset(e2[:, :], 0.0)
            nc.sync.dma_start(out=e2[2:P, 0:1], in_=s[0 : P - 2, W - 1 : W])
            nc.vector.scalar_tensor_tensor(
                out=ecol[:, 0:1], in0=e2[:, 0:1], scalar=cF, in1=ecol[:, 0:1],
                op0=Alu.mult, op1=Alu.add,
            )
            self.rels(e2)
        # scan2 (vector): chained scan
        nc.vector.tensor_tensor_scan(
            out=s[:, C0:W], data0=cb[:, 0:1].broadcast_to([P, CH]),
            data1=xt[:, C0:W], initial=ecol[:, 0:1], op0=Alu.mult, op1=Alu.add,
        )
        self.rels(cb, ecol)
        # halo fill (partition 0 halo zeroed)
        nc.vector.memset(s[0:1, 0:HP], 0.0)
        nc.sync.dma_start(out=s[1:P, 0:HP], in_=s[0 : P - 1, CH:W])
        # ss = -a*scale*s on ACT over full width
        ss = self.big()
        nc.scalar.mul(ss[:, 0:W], s[:, 0:W], -a * scale)
        self.rel(s)
        y = self.big()
        if K <= 248:
            nc.vector.scalar_tensor_tensor(
                out=y[:, 248:W], in0=ss[:, 248 - K : W - K], scalar=cK,
                in1=ss[:, 248:W], op0=Alu.mult, op1=Alu.subtract,
            )
        else:
            sk = self.big()
            nc.vector.memset(sk[0:1, :], 0.0)
            # piece A: cols [248, K) from partition p-1
            nc.sync.dma_start(
                out=sk[1:P, 248:K], in_=ss[0 : P - 1, 248 + CH - K : CH]
            )
            # piece B: cols [K, W) from same partition
            nc.sync.dma_start(out=sk[:, K:W], in_=ss[:, 0 : W - K])
            nc.vector.scalar_tensor_tensor(
                out=y[:, 248:W], in0=sk[:, 248:W], scalar=cK, in1=ss[:, 248:W],
                op0=Alu.mult, op1=Alu.subtract,
            )
            self.rel(sk)
        self.rel(ss)
        return y

    def winmaxmin(self, xt, n, op):
        """sliding window max (op=Alu.max) or min over window n.
        xt valid on [0, W) (loaded inputs). returns tile valid [n-1, W)."""
        nc = self.nc
        J = int(math.floor(math.log2(n)))
        r = n - (1 << J)
        cur = xt
        curlo = 0
        scratch = []
        for j in range(J):
            sh = 1 << j
            dst = self.big()
            scratch.append(dst)
            nc.vector.tensor_tensor(
                out=dst[:, curlo + sh : W], in0=cur[:, curlo + sh : W],
                in1=cur[:, curlo : W - sh], op=op,
            )
            cur = dst
            curlo = curlo + sh
        out = self.big()
        if r > 0:
            nc.vector.tensor_tensor(
                out=out[:, curlo + r : W], in0=cur[:, curlo + r : W],
                in1=cur[:, curlo : W - r], op=op,
            )
        else:
            nc.vector.tensor_copy(out[:, curlo:W], cur[:, curlo:W])
        for t in scratch:
            self.rel(t)
        return out

    def winsum18(self, xt, xlo):
        """rolling 18-window sum of xt (valid from xlo); returns tile valid
        [xlo+31, W)."""
        nc = self.nc
        tiles = []
        cur = xt
        curlo = xlo
        chain1 = None
        for j in range(4):
            sh = 1 << j
            dst = self.big()
            tiles.append(dst)
            nc.vector.tensor_add(
                dst[:, curlo + sh : W], cur[:, curlo + sh : W], cur[:, curlo : W - sh]
            )
            cur = dst
            curlo += sh
            if j == 0:
                chain1 = dst  # window-2 sums
        out = self.big()
        # S18[t] = W16[t] + W2[t-16]
        lo = curlo + 16
        nc.vector.tensor_add(out[:, lo:W], cur[:, lo:W], chain1[:, lo - 16 : W - 16])
        for t in tiles:
            self.rel(t)
        return out, lo

    # ---- full pipeline ----
    def build(self):
        nc = self.nc
        with ctile.TileContext(nc) as tc:
            with tc.tile_pool(name="big", bufs=1) as pool, tc.tile_pool(
                name="small", bufs=1
            ) as spool:
                self.pool = pool
                self.spool = spool
                self.emit()
        nc.finalize()
        return nc

    def emit(self):
        nc = self.nc
        self.eps8 = self.spool.tile([P, 1], F32, tag="c_eps8")
        nc.vector.memset(self.eps8[:, :], 1e-8)
        self.nanch = self.spool.tile([P, 1], F32, tag="c_nanch")
        nc.vector.memset(self.nanch[:, :], -self.anchor)
        Ct = self.load_series(self.DC)
        Ht = self.load_series(self.DH)
        Lt = self.load_series(self.DL)
        self.store_row(0, Ct)
        self.store_row(1, Ht)
        self.store_row(2, Lt)

        # --- TEMA3 chain ---
        EMA1 = self.ema(Ct, 0)
        EMA2 = self.ema(EMA1, 1)
        EMA3 = self.ema(EMA2, 2)
        TEMA3 = self.big()
        d = self.big()
        nc.vector.tensor_sub(d[:, 248:W], EMA1[:, 248:W], EMA2[:, 248:W])
        nc.vector.scalar_tensor_tensor(
            out=TEMA3[:, 248:W], in0=d[:, 248:W], scalar=3.0, in1=EMA3[:, 248:W],
            op0=Alu.mult, op1=Alu.add,
        )
        self.rel(EMA1, EMA2, EMA3, d)
        self.store_row(4, TEMA3)

        # --- TEMAP2 chain ---
        E21 = self.ema(Ct, 3)
        E221 = self.ema(E21, 4)
        E231 = self.ema(E221, 5)
        TEMAP2 = self.big()
        d = self.big()
        nc.vector.tensor_sub(d[:, 248:W], E21[:, 248:W], E221[:, 248:W])
        nc.vector.scalar_tensor_tensor(
            out=TEMAP2[:, 248:W], in0=d[:, 248:W], scalar=3.0, in1=E231[:, 248:W],
            op0=Alu.mult, op1=Alu.add,
        )
        self.rel(E21, E221, E231, d)

        # --- T ratios ---
        def tdiff(xt, lag, row_idx, lo=254):
            dt_ = self.big()
            nc.vector.tensor_tensor(
                out=dt_[:, lo:W], in0=xt[:, lo:W], in1=xt[:, lo - lag : W - lag],
                op=Alu.subtract,
            )
            ab = self.big()
            nc.scalar.activation(ab[:, lo:W], xt[:, lo - lag : W - lag], Act.Abs)
            abe = self.big()
            nc.scalar.activation(abe[:, lo:W], ab[:, lo:W], Act.Identity, bias=self.eps8[:, 0:1])
            rr = self.big()
            nc.vector.reciprocal(rr[:, lo:W], abe[:, lo:W])
            ts_ = self.big()
            nc.vector.tensor_mul(ts_[:, lo:W], dt_[:, lo:W], rr[:, lo:W])
            self.rel(dt_, ab, abe, rr)
            if row_idx is not None:
                self.store_row(row_idx, ts_)
            return ts_

        T3s = tdiff(TEMA3, 6, 8)
        T1s = tdiff(TEMA3, 1, 6)
        T2s = tdiff(TEMAP2, 6, 7)
        self.rel(TEMAP2)

        # --- stdp(C,18) anchored at global C[0] ---
        dev = self.big()
        nc.scalar.activation(dev[:, 0:W], Ct[:, 0:W], Act.Identity, bias=self.nanch[:, 0:1])
        dev2 = self.big()
        nc.vector.tensor_mul(dev2[:, 0:W], dev[:, 0:W], dev[:, 0:W])
        S18, lo1 = self.winsum18(dev, 0)
        Q18, lo2 = self.winsum18(dev2, 0)
        self.rel(dev, dev2)
        m = self.big()
        nc.scalar.mul(m[:, lo1:W], S18[:, lo1:W], 1.0 / 18.0)
        ex2 = self.big()
        nc.scalar.mul(ex2[:, lo2:W], Q18[:, lo2:W], 1.0 / 18.0)
        self.rel(S18, Q18)
        mm = self.big()
        nc.vector.tensor_mul(mm[:, lo1:W], m[:, lo1:W], m[:, lo1:W])
        var = self.big()
        nc.vector.tensor_sub(var[:, lo1:W], ex2[:, lo1:W], mm[:, lo1:W])
        nc.vector.tensor_scalar_max(var[:, lo1:W], var[:, lo1:W], 0.0)
        DIS = self.big()
        nc.scalar.activation(DIS[:, lo1:W], var[:, lo1:W], Act.Sqrt)
        self.rel(m, ex2, mm, var)
        TEU3 = self.big()
        nc.vector.tensor_add(TEU3[:, C0:W], TEMA3[:, C0:W], DIS[:, C0:W])
        TED = self.big()
        nc.vector.tensor_sub(TED[:, C0:W], TEMA3[:, C0:W], DIS[:, C0:W])
        self.store_row(3, TEU3)
        self.store_row(5, TED)
        self.rel(TEMA3, DIS, TEU3, TED)

        # --- KDJ blocks ---
        def kdj(nw, ik, idd, rows):
            hh = self.winmaxmin(Ht, nw, Alu.max)
            ll = self.winmaxmin(Lt, nw, Alu.min)
            hl = self.big()
            nc.vector.tensor_sub(hl[:, C0:W], hh[:, C0:W], ll[:, C0:W])
            nc.vector.tensor_scalar_max(hl[:, C0:W], hl[:, C0:W], 1e-8)
            rcp = self.big()
            nc.vector.reciprocal(rcp[:, C0:W], hl[:, C0:W])
            num = self.big()
            nc.vector.tensor_sub(num[:, C0:W], Ct[:, C0:W], ll[:, C0:W])
            r0 = self.big()
            nc.vector.tensor_mul(r0[:, C0:W], num[:, C0:W], rcp[:, C0:W])
            self.rel(hh, ll, hl, rcp, num)
            rsv01 = self.big()
            nc.vector.tensor_scalar(
                out=rsv01[:, C0:W], in0=r0[:, C0:W], scalar1=0.0, scalar2=1.0,
                op0=Alu.max, op1=Alu.min,
            )
            self.rel(r0)
            Kv = self.ema(rsv01, ik, scale=100.0)
            self.rel(rsv01)
            Dv = self.ema(Kv, idd)
            Jv = self.big()
            dkd = self.big()
            nc.vector.tensor_sub(dkd[:, 248:W], Kv[:, 248:W], Dv[:, 248:W])
            nc.vector.scalar_tensor_tensor(
                out=Jv[:, 248:W], in0=dkd[:, 248:W], scalar=2.0, in1=Kv[:, 248:W],
                op0=Alu.mult, op1=Alu.add,
            )
            self.rel(dkd)
            for ridx, tt_ in zip(rows, (Kv, Dv, Jv)):
                if ridx is not None:
                    self.store_row(ridx, tt_)
            return Kv, Dv, Jv

        K1, D1, J1 = kdj(204, 6, 7, (9, 10, 11))
        self.rel(K1, D1)
        K2, D2, J2 = kdj(18, 8, 9, (12, 13, 14))
        self.rel(K2, D2)
        K3, D3, J3 = kdj(9, 10, 11, (15, 16, 17))
        self.rel(K3, D3)
        KN3, DN3, JN3 = kdj(36, 12, 13, (None, None, 18))
        self.rel(KN3, DN3, JN3)
        self.rel(Ht, Lt)

        # --- JX family ---
        JXb = self.big()
        u = self.big()
        nc.vector.tensor_mul(u[:, 254:W], J3[:, 254:W], T1s[:, 254:W])
        v = self.big()
        nc.vector.tensor_add(v[:, 254:W], J1[:, 254:W], J2[:, 254:W])
        nc.vector.tensor_add(JXb[:, 254:W], u[:, 254:W], v[:, 254:W])
        self.rel(u, v, J3, T1s)
        F1 = self.big()
        nc.vector.tensor_mul(F1[:, 254:W], J2[:, 254:W], T3s[:, 254:W])
        self.rel(J2, T3s)
        F2 = self.big()
        nc.vector.tensor_mul(F2[:, 254:W], J1[:, 254:W], T2s[:, 254:W])
        self.rel(J1, T2s)
        self.store_row(19, JXb)
        self.store_row(20, F1)
        self.store_row(21, F2)

        EMA_JX = self.ema(JXb, 14)
        EMA_F1 = self.ema(F1, 15)
        EMA_F2 = self.ema(F2, 16)
        EMA8_JX = self.ema(JXb, 17)
        EMA8_F1 = self.ema(F1, 18)
        EMA8_F2 = self.ema(F2, 19)
        self.store_row(22, EMA_JX)
        self.store_row(23, EMA_F1)
        self.store_row(24, EMA_F2)

        def jx_combine(base, f1, f2, row_idx, lo=254):
            w_ = self.big()
            nc.vector.tensor_add(w_[:, lo:W], f1[:, lo:W], f2[:, lo:W])
            z = self.big()
            nc.vector.scalar_tensor_tensor(
                out=z[:, lo:W], in0=w_[:, lo:W], scalar=6.0, in1=base[:, lo:W],
                op0=Alu.mult, op1=Alu.add,
            )
            out = self.big()
            nc.vector.tensor_scalar_sub(out[:, lo:W], z[:, lo:W], 50.0)
            self.rel(w_, z)
            self.store_row(row_idx, out)
            return out

        # need col 255 for the cross lag -> compute from col 254
        JX = jx_combine(JXb, F1, F2, 27, lo=254)
        EMAJX = jx_combine(EMA_JX, EMA_F1, EMA_F2, 28, lo=254)
        EMAJX8 = jx_combine(EMA8_JX, EMA8_F1, EMA8_F2, 29, lo=254)
        self.rel(JXb, F1, F2, EMA_JX, EMA_F1, EMA_F2, EMA8_JX, EMA8_F1, EMA8_F2)
        self.rel(EMAJX8)

        # --- crosses + segmented MA scans ---
        def macond(updown):
            g = self.big()
            l = self.big()
            if updown == "up":
                nc.vector.tensor_tensor(
                    out=g[:, 255:W], in0=JX[:, 255:W], in1=EMAJX[:, 255:W],
                    op=Alu.is_gt,
                )
                nc.vector.tensor_tensor(
                    out=l[:, 255:W], in0=JX[:, 254 : W - 1],
                    in1=EMAJX[:, 254 : W - 1], op=Alu.is_le,
                )
            else:
                nc.vector.tensor_tensor(
                    out=g[:, 255:W], in0=JX[:, 255:W], in1=EMAJX[:, 255:W],
                    op=Alu.is_lt,
                )
                nc.vector.tensor_tensor(
                    out=l[:, 255:W], in0=JX[:, 254 : W - 1],
                    in1=EMAJX[:, 254 : W - 1], op=Alu.is_ge,
                )
            cond = self.big()
            nc.vector.tensor_mul(cond[:, 255:W], g[:, 255:W], l[:, 255:W])
            m_ = self.big()
            nc.vector.tensor_scalar(
                out=m_[:, 255:W], in0=cond[:, 255:W], scalar1=-1.0, scalar2=1.0,
                op0=Alu.mult, op1=Alu.add,
            )
            self.rel(g, l)
            return cond, m_

        # Manual expansion (cnt first to derive A, then S and seen share it)
        for updown, row_idx, diag_idx in (("dn", 25, 1), ("up", 26, 0)):
            cond, m_ = macond(updown)
            dmask = self.big()
            nc.vector.tensor_mul(dmask[:, C0:W], Ct[:, C0:W], m_[:, C0:W])

            # --- cnt: scan1 on gpsimd, extract A row, chain, scan2 ---
            cnt_s = self.big()
            nc.vector.tensor_tensor_scan(
                out=cnt_s[:, C0:W], data0=m_[:, C0:W], data1=m_[:, C0:W],
                initial=0.0, op0=Alu.mult, op1=Alu.add,
            )
            acol = self.small()
            nc.vector.tensor_single_scalar(
                out=acol[:, 0:1], in_=cnt_s[:, W - 1 : W], scalar=float(CH),
                op=Alu.is_ge,
            )
            arow = self.row()
            nc.sync.dma_start(out=arow[0:1, 0:P], in_=acol[:, 0:1])
            self.rels(acol)

            def chain(scan1_tile, use_arow, op0, op1, d0, d1):
                brow = self.row()
                nc.sync.dma_start(out=brow[0:1, 0:P], in_=scan1_tile[:, W - 1 : W])
                crow = self.row()
                if use_arow is None:
                    nc.vector.tensor_tensor_scan(
                        out=crow[0:1, 0:P], data0=brow[0:1, 0:P],
                        data1=brow[0:1, 0:P], initial=0.0, op0=Alu.max, op1=Alu.max,
                    )
                else:
                    nc.vector.tensor_tensor_scan(
                        out=crow[0:1, 0:P], data0=use_arow[0:1, 0:P],
                        data1=brow[0:1, 0:P], initial=0.0, op0=Alu.mult, op1=Alu.add,
                    )
                ccol = self.small()
                nc.vector.memset(ccol[:, :], 0.0)
                nc.sync.dma_start(out=ccol[1:P, 0:1], in_=crow[0:1, 0 : P - 1])
                self.relr(brow, crow)
                nc.vector.tensor_tensor_scan(
                    out=scan1_tile[:, C0:W], data0=d0, data1=d1,
                    initial=ccol[:, 0:1], op0=op0, op1=op1,
                )
                self.rels(ccol)

            chain(cnt_s, arow, Alu.mult, Alu.add, m_[:, C0:W], m_[:, C0:W])

            # --- S: sum since last event ---
            Ssum = self.big()
            nc.vector.tensor_tensor_scan(
                out=Ssum[:, C0:W], data0=m_[:, C0:W], data1=dmask[:, C0:W],
                initial=0.0, op0=Alu.mult, op1=Alu.add,
            )
            chain(Ssum, arow, Alu.mult, Alu.add, m_[:, C0:W], dmask[:, C0:W])

            # --- seen: running max of cond ---
            seen = self.big()
            nc.vector.tensor_tensor_scan(
                out=seen[:, C0:W], data0=cond[:, C0:W], data1=cond[:, C0:W],
                initial=0.0, op0=Alu.max, op1=Alu.max,
            )
            chain(seen, None, Alu.max, Alu.max, cond[:, C0:W], cond[:, C0:W])
            self.relr(arow)
            self.rel(cond, m_, dmask)

            # ma = (S * recip(max(cnt,1))) * seen
            rc = self.big()
            nc.vector.tensor_scalar_max(rc[:, C0:W], cnt_s[:, C0:W], 1.0)
            rcp = self.big()
            nc.vector.reciprocal(rcp[:, C0:W], rc[:, C0:W])
            ma0 = self.big()
            nc.vector.tensor_mul(ma0[:, C0:W], Ssum[:, C0:W], rcp[:, C0:W])
            ma = self.big()
            nc.vector.tensor_mul(ma[:, C0:W], ma0[:, C0:W], seen[:, C0:W])
            self.rel(rc, rcp, ma0, cnt_s, Ssum)
            self.store_row(row_idx, ma)

            # diag: min of seen over partitions 5..127 (covers valid region)
            dcol = self.small()
            nc.vector.tensor_reduce(
                out=dcol[0:P, 0:1], in_=seen[0:P, C0:W],
                axis=mybir.AxisListType.X, op=Alu.min,
            )
            drow = self.row()
            nc.sync.dma_start(out=drow[0:1, 0 : P - 5], in_=dcol[5:P, 0:1])
            done = self.spool.tile([1, 1], F32, tag=f"diag{diag_idx}")
            nc.vector.tensor_reduce(
                out=done[0:1, 0:1], in_=drow[0:1, 0 : P - 5],
                axis=mybir.AxisListType.X, op=Alu.min,
            )
            self.relr(drow)
            nc.sync.dma_start(
                out=self.DIAG[diag_idx : diag_idx + 1].rearrange(
                    "(a b) -> a b", a=1, b=1
                ),
                in_=done[0:1, 0:1],
            )
            self.rels(dcol)
            self.rel(seen, ma)

        self.rel(Ct, JX, EMAJX)


_CACHE = {}


def _build(alphas, anchor):
    key = (tuple(round(float(a), 12) for a in alphas), round(float(anchor), 6))
    if key not in _CACHE:
        kb = KB(alphas, anchor)
        _CACHE[key] = kb.build()
    return _CACHE[key]


def _shard(x):
    """per-core input arrays [DLEN], clamp-padded on the global left."""
    outs = []
    for mcore in range(NCORES):
        lo = (mcore + 1) * S - DLEN
        if lo < 0:
            d = np.concatenate(
                [np.full(-lo, x[0], np.float32), x[0 : (mcore + 1) * S]]
            )
        else:
            d = x[lo : (mcore + 1) * S]
        outs.append(np.ascontiguousarray(d, np.float32))
    return outs


def _host_ma(C, JX, EJ):
    """exact host fallback for ma rows (numpy, global)."""
    f32 = np.float32
    T_ = len(C)
    lag = lambda x: np.concatenate([x[:1], x[:-1]])
    JXp, EJp = lag(JX), lag(EJ)
    res = {}
    cs = np.concatenate([[0.0], np.cumsum(C.astype(np.float64))])
    t_idx = np.arange(T_)
    for key, cond in (
        ("dn", (JX < EJ) & (JXp >= EJp)),
        ("up", (JX > EJ) & (JXp <= EJp)),
    ):
        last = np.maximum.accumulate(np.where(cond, t_idx, -1))
        csl = cs[np.maximum(last, 0) + 1]
        s = cs[t_idx + 1] - csl
        n = t_idx - last
        res[key] = np.where(
            (last >= 0) & (n > 0), s / np.maximum(n, 1), 0.0
        ).astype(f32)
    return res["dn"], res["up"]


def run_cores(inputs, trace=False):
    """compile (cached) + run on 8 cores; returns (results, BassKernelResults)."""
    C = np.ascontiguousarray(inputs["C"], np.float32)
    H = np.ascontiguousarray(inputs["H"], np.float32)
    L = np.ascontiguousarray(inputs["L"], np.float32)
    w = np.asarray(inputs["w_alphas"], np.float32)
    alphas = [float(1.0 / (1.0 + math.exp(-float(x)))) for x in w]
    nc = _build(alphas, float(C[0]))
    dc, dh, dl = _shard(C), _shard(H), _shard(L)
    in_maps = [
        {"DC": dc[m], "DH": dh[m], "DL": dl[m]} for m in range(NCORES)
    ]
    res = run_bass_kernel_spmd(
        nc, in_maps, core_ids=list(range(NCORES)), trace=trace
    )
    return res


def kernel(C, H, L, w_alphas):
    inputs = {"C": C, "H": H, "L": L, "w_alphas": w_alphas}
    res = run_cores(inputs)
    outs = [res.results[m]["OUT"].reshape(NROWS, S) for m in range(NCORES)]
    full = np.concatenate(outs, axis=1)

    # host patch: reference's partial-window std for the first 17 bars
    Cg = np.asarray(C, np.float64)[:17]
    for t in range(17):
        wdw = Cg[: t + 1]
        dis = math.sqrt(max(np.mean(wdw * wdw) - np.mean(wdw) ** 2, 0.0))
        full[3, t] = np.float32(full[4, t] + dis)
        full[5, t] = np.float32(full[4, t] - dis)

    # diag check: cross gap exceeded the halo on some core -> exact host fix
    need_fix = False
    for mcore in range(1, NCORES):
        dg = res.results[mcore]["DIAG"]
        if dg.min() < 0.5:
            need_fix = True
    if need_fix:
        ma_dn, ma_up = _host_ma(
            np.asarray(C, np.float32), full[27], full[28]
        )
        full[25] = ma_dn
        full[26] = ma_up
    return full.astype(np.float32)
